# revision 1
# baseline (speedup 1.0000x reference)
"""Chamfer distance kernel for Trainium2 (8 NeuronCores).

Problem: B=4 batches of x[8192,3], y[8192,3] float32. For each batch:
  dist[n,m] = clip(||x_n||^2 + ||y_m||^2 - 2<x_n,y_m> + EPS, EPS, 1e9)
  outputs: rowwise min+argmin (x->y) and colwise min+argmin (y->x).

Sharding: one core per (batch, direction) = 4*2 = 8 cores. Each core
computes a full 8192x8192 distance block with its own "stationary" (row)
and "moving" (column) point set and reduces over the moving dim, so no
cross-core combine is needed.

Per core the distance matrix is produced by a single K=33 matmul whose
contraction folds in the squared norms: both operands are decomposed into
3 bf16 components per value (hi/mid/lo), all 9 cross products per
coordinate are included, so the PE computes -(dist - EPS) to ~2^-26
relative accuracy at full bf16 rate. EPS and the clip are applied to the
final per-row minima afterwards (monotonic, so the argmin is unchanged).

Per 128-row block: PE matmuls -> PSUM, ACT copies PSUM->SBUF, DVE does a
fused max+accumulate (row max of the negated distance = min distance) and
a max_index pass (first-occurrence argmax = reference argmin tie-break).
"""

import sys

for _p in ("/opt/trn_rl_repo", "/root/.axon_site/_ro/trn_rl_repo"):
    if _p not in sys.path:
        sys.path.append(_p)

import numpy as np
import ml_dtypes

import concourse.bacc as bacc
import concourse.mybir as mybir
from concourse.tile import TileContext
from concourse.bass_utils import run_bass_kernel_spmd

BF16 = ml_dtypes.bfloat16
F32 = mybir.dt.float32
BF = mybir.dt.bfloat16
U32 = mybir.dt.uint32

EPS = np.float32(1e-8)
DIST_MAX = np.float32(1.0e9)
SANITIZE_CLIP = 1.0e5

B = 4
NPTS = 8192          # both N and M
P = 128              # partitions / rows per block
NBLK = NPTS // P     # 64 row blocks
CHUNK = 512          # matmul free dim (one PSUM bank of f32)
GROUP = 2048         # ACT copy granularity (4 banks)
NGROUP = NPTS // GROUP
KROWS = 33           # 27 product rows + 3 + 3 norm rows

_NC_CACHE = {}


def _build_nc(npts_s, npts_m):
    """Build the SPMD program: one (stationary x moving) block per core.

    npts_s: number of stationary points (rows, multiple of 128)
    npts_m: number of moving points (cols, multiple of GROUP)
    """
    nblk = npts_s // P
    ngroup = npts_m // GROUP

    nc = bacc.Bacc("TRN2", target_bir_lowering=False, debug=False, num_devices=8)
    w_d = nc.dram_tensor("W", [KROWS, npts_s], BF, kind="ExternalInput")
    r_d = nc.dram_tensor("R", [KROWS, npts_m], BF, kind="ExternalInput")
    ov_d = nc.dram_tensor("OV", [P, nblk], F32, kind="ExternalOutput")
    oi_d = nc.dram_tensor("OI", [P, nblk], U32, kind="ExternalOutput")

    with TileContext(nc) as tc:
        with (
            tc.tile_pool(name="const", bufs=1) as cpool,
            tc.tile_pool(name="work", bufs=2) as wpool,
            tc.tile_pool(name="psum", bufs=2, space="PSUM") as ppool,
            tc.tile_pool(name="small", bufs=3) as spool,
            tc.tile_pool(name="outacc", bufs=1) as opool,
        ):
            wt = cpool.tile([KROWS, npts_s], BF)
            rt = cpool.tile([KROWS, npts_m], BF)
            nc.gpsimd.dma_start(wt[:], w_d[:])
            nc.gpsimd.dma_start(rt[:], r_d[:])

            zeros8 = cpool.tile([P, 8], F32)
            nc.gpsimd.memset(zeros8[:], 0.0)

            ovals = opool.tile([P, nblk], F32)
            oidx = opool.tile([P, nblk], U32)

            for blk in range(nblk):
                rb = wpool.tile([P, npts_m], F32, tag="rowbuf")
                for g in range(ngroup):
                    pt = ppool.tile([P, GROUP], F32, tag="pgroup")
                    for cc in range(GROUP // CHUNK):
                        f0 = g * GROUP + cc * CHUNK
                        nc.tensor.matmul(
                            pt[:, cc * CHUNK:(cc + 1) * CHUNK],
                            wt[:, blk * P:(blk + 1) * P],
                            rt[:, f0:f0 + CHUNK],
                            start=True,
                            stop=True,
                        )
                    nc.scalar.activation(
                        rb[:, g * GROUP:(g + 1) * GROUP],
                        pt[:],
                        mybir.ActivationFunctionType.Copy,
                    )

                # row max of negated distance (fused reduce), 2x fp32 mode
                m_ = spool.tile([P, 1], F32, tag="m")
                nc.vector.tensor_scalar(
                    rb[:],
                    rb[:],
                    scalar1=-3.0e38,
                    scalar2=None,
                    op0=mybir.AluOpType.max,
                    accum_out=m_[:],
                    op1=mybir.AluOpType.max,
                )
                # broadcast m across 8 lanes for max_index's in_max
                m8 = spool.tile([P, 8], F32, tag="m8")
                nc.vector.tensor_scalar(
                    m8[:],
                    zeros8[:],
                    scalar1=m_[:],
                    scalar2=None,
                    op0=mybir.AluOpType.add,
                )
                i8 = spool.tile([P, 8], U32, tag="i8")
                nc.vector.max_index(i8[:], m8[:], rb[:])
                nc.vector.tensor_copy(oidx[:, blk:blk + 1], i8[:, 0:1])

                # min dist = clip(-m + EPS, EPS, DIST_MAX)
                v1 = spool.tile([P, 1], F32, tag="v1")
                nc.vector.tensor_scalar(
                    v1[:],
                    m_[:],
                    scalar1=-1.0,
                    scalar2=float(EPS),
                    op0=mybir.AluOpType.mult,
                    op1=mybir.AluOpType.add,
                )
                nc.vector.tensor_scalar(
                    ovals[:, blk:blk + 1],
                    v1[:],
                    scalar1=float(EPS),
                    scalar2=float(DIST_MAX),
                    op0=mybir.AluOpType.max,
                    op1=mybir.AluOpType.min,
                )

            nc.sync.dma_start(ov_d[:], ovals[:])
            nc.sync.dma_start(oi_d[:], oidx[:])

    nc.compile()
    return nc


def _split3(a):
    """f32 array -> three bf16 components (hi, mid, lo), a ~= h+m+l."""
    a = np.asarray(a, np.float32)
    h = a.astype(BF16)
    r1 = a - h.astype(np.float32)
    m = r1.astype(BF16)
    r2 = r1 - m.astype(np.float32)
    l = r2.astype(BF16)
    return h, m, l


def _make_operands(s_pts, m_pts):
    """Build W [33, Ns] (stationary) and R [33, Nm] (moving) bf16 matrices.

    out[p, f] = sum_k W[k, p] * R[k, f]
             ~= 2<s_p, m_f> - ||s_p||^2 - ||m_f||^2  (= EPS-less -dist)
    """
    s_pts = np.asarray(s_pts, np.float32)
    m_pts = np.asarray(m_pts, np.float32)
    ns, nm = s_pts.shape[0], m_pts.shape[0]
    w = np.zeros((KROWS, ns), BF16)
    r = np.zeros((KROWS, nm), BF16)
    k = 0
    for d in range(3):
        sh, sm, sl = _split3(s_pts[:, d])
        th, tm, tl = _split3(m_pts[:, d])
        s2h = (2.0 * sh.astype(np.float32)).astype(BF16)
        s2m = (2.0 * sm.astype(np.float32)).astype(BF16)
        s2l = (2.0 * sl.astype(np.float32)).astype(BF16)
        for scomp in (s2h, s2m, s2l):
            for tcomp in (th, tm, tl):
                w[k] = scomp
                r[k] = tcomp
                k += 1
    # -||s||^2 against ones
    s_sq = np.sum(s_pts * s_pts, axis=1, dtype=np.float32)
    qh, qm, ql = _split3(-s_sq)
    for comp in (qh, qm, ql):
        w[k] = comp
        r[k] = BF16(1.0)
        k += 1
    # -||m||^2 against ones
    m_sq = np.sum(m_pts * m_pts, axis=1, dtype=np.float32)
    rh, rm, rl = _split3(-m_sq)
    for comp in (rh, rm, rl):
        w[k] = BF16(1.0)
        r[k] = comp
        k += 1
    assert k == KROWS
    return w, r


def kernel(input1, input2):
    x = np.nan_to_num(
        np.asarray(input1, np.float32),
        nan=0.0, posinf=SANITIZE_CLIP, neginf=-SANITIZE_CLIP,
    )
    y = np.nan_to_num(
        np.asarray(input2, np.float32),
        nan=0.0, posinf=SANITIZE_CLIP, neginf=-SANITIZE_CLIP,
    )
    assert x.shape == (B, NPTS, 3) and y.shape == (B, NPTS, 3)

    if "nc" not in _NC_CACHE:
        _NC_CACHE["nc"] = _build_nc(NPTS, NPTS)
    nc = _NC_CACHE["nc"]

    in_maps = []
    for c in range(8):
        b, d = c // 2, c % 2
        s_pts, m_pts = (x[b], y[b]) if d == 0 else (y[b], x[b])
        w, r = _make_operands(s_pts, m_pts)
        in_maps.append({"W": w, "R": r})

    res = run_bass_kernel_spmd(nc, in_maps, list(range(8)))

    min_x2y = np.empty((B, NPTS), np.float32)
    min_y2x = np.empty((B, NPTS), np.float32)
    idx1 = np.empty((B, NPTS), np.int32)
    idx2 = np.empty((B, NPTS), np.int32)
    for c in range(8):
        b, d = c // 2, c % 2
        vals = np.asarray(res.results[c]["OV"]).T.reshape(-1)
        idxs = np.asarray(res.results[c]["OI"]).astype(np.int64).T.reshape(-1)
        if d == 0:
            min_x2y[b] = vals
            idx1[b] = idxs.astype(np.int32)
        else:
            min_y2x[b] = vals
            idx2[b] = idxs.astype(np.int32)
    return min_x2y, min_y2x, idx1, idx2
